# revision 12
# baseline (speedup 1.0000x reference)
import sys

sys.path.insert(0, "/opt/trn_rl_repo")

import math

import ml_dtypes
import numpy as np
import concourse.bass as bass  # noqa: F401  (registers types)
from concourse import bacc
import concourse.mybir as mybir
from concourse.tile import TileContext
from concourse.bass_utils import run_bass_kernel_spmd

S = 4096          # sequence length
D = 1024          # model/key/value dim
NCORES = 8
R = S // NCORES   # 512 query rows per core
KC = D // 128     # 8 contraction chunks
NF = D // 128     # 8 feature chunks
J = S // 128      # 32 key tiles
VA = D + 16       # V augmented with ones col (denominator); padded so the
                  # DoubleRow weight AP dim1 step stays %16==0
SC = 1.0 / math.sqrt(D)

F32 = mybir.dt.float32
I32 = mybir.dt.int32
BF16 = mybir.dt.bfloat16
F8 = mybir.dt.float8e4
DR = mybir.MatmulPerfMode.DoubleRow
Exp = mybir.ActivationFunctionType.Exp
Ident = mybir.ActivationFunctionType.Identity
NP_F8 = ml_dtypes.float8_e4m3

# phase-2 kt DMA groups: ramp up so the first matmul waits on 128KB only
KT_GROUPS = [[0], [1, 2, 3]] + [list(range(4 * g, 4 * g + 4)) for g in range(1, 8)]
# queue routing: early + late-needed groups ride the scalar queue, the
# mid-stage groups the sync queue (each queue sustains ~100GB/s)
KT_ON_SCALAR = {0, 1, 6, 7, 8}

_cache = {}
# test.py can flip TRACE to get exec_time_ns of the two launches in LAST_NS
TRACE = False
LAST_NS = None
PHASE_NS = None


def _build_phase1():
    """Per core: q^T+bq, k^T+bk (transposed, contraction-chunked, fp8 out) and
    v+bv (natural, fp8) for its 512-row x slice. fp8 DoubleRow matmuls,
    512-wide moving output. Inputs/outputs spread over the sync, scalar and
    gpsimd DMA queues (~100GB/s each); compute order q, v, k.

      xsT [128, KC, R]: [p, k, i] = x[512c+i, 128k+p]
      wq/wk [128, NF*KC, 128]: [p, f*KC+k, c] = W[128k+p, 128f+c]
      wv [128, KC, D]: [p, k, d] = W[128k+p, d]
      bqk [128, 2, NF] f32: [p, 0/1, f] = bq/bk[128f+p]
      bvt [128, D] bf16: bv broadcast to all partitions
    Outputs (all biased on device, single fp8 quantization):
      qT/kT [128, KC, R] fp8: [p, f, i] = (x@W + b)^T[128f+p, i]
      vO [R, D] fp8 natural.
    """
    nc = bacc.Bacc(None, target_bir_lowering=False)
    xsT = nc.dram_tensor("xsT", [128, KC, R], F8, kind="ExternalInput")
    wq = nc.dram_tensor("wq", [128, NF * KC, 128], F8, kind="ExternalInput")
    wk = nc.dram_tensor("wk", [128, NF * KC, 128], F8, kind="ExternalInput")
    wv = nc.dram_tensor("wv", [128, KC, D], F8, kind="ExternalInput")
    bqk = nc.dram_tensor("bqk", [128, 2, NF], F32, kind="ExternalInput")
    bvt = nc.dram_tensor("bvt", [128, D], BF16, kind="ExternalInput")
    qT = nc.dram_tensor("qT", [128, KC, R], F8, kind="ExternalOutput")
    kT = nc.dram_tensor("kT", [128, KC, R], F8, kind="ExternalOutput")
    vO = nc.dram_tensor("vO", [R, D], F8, kind="ExternalOutput")
    with TileContext(nc) as tc:
        with tc.tile_pool(name="inp", bufs=1) as inp, \
             tc.tile_pool(name="ob", bufs=1) as ob, \
             tc.tile_pool(name="ps", bufs=4, space="PSUM") as ps:
            bqkt = inp.tile([128, 2, NF], F32)
            bvtt = inp.tile([128, D], BF16)
            xt = inp.tile([128, KC, R], F8)
            wqt = inp.tile([128, NF * KC, 128], F8)
            wkt = inp.tile([128, NF * KC, 128], F8)
            wvt = inp.tile([128, KC, D], F8)
            # sync queue: x, Wq, Wv feature-half 0
            nc.sync.dma_start(xt[:, 0:2, :], xsT[:, 0:2, :])
            nc.sync.dma_start(wqt[:, 0:2 * KC, :], wq[:, 0:2 * KC, :])
            nc.sync.dma_start(xt[:, 2:4, :], xsT[:, 2:4, :])
            nc.sync.dma_start(xt[:, 4:8, :], xsT[:, 4:8, :])
            nc.sync.dma_start(wqt[:, 2 * KC:4 * KC, :], wq[:, 2 * KC:4 * KC, :])
            nc.sync.dma_start(wqt[:, 4 * KC:NF * KC, :], wq[:, 4 * KC:NF * KC, :])
            nc.sync.dma_start(wvt[:, :, 0:512], wv[:, :, 0:512])
            # scalar queue: biases, Wv feature-half 1, Wk
            nc.scalar.dma_start(bqkt[:], bqk[:])
            nc.scalar.dma_start(bvtt[:], bvt[:])
            nc.scalar.dma_start(wvt[:, :, 512:D], wv[:, :, 512:D])
            nc.scalar.dma_start(wkt[:, 0:4 * KC, :], wk[:, 0:4 * KC, :])
            nc.scalar.dma_start(wkt[:, 4 * KC:NF * KC, :], wk[:, 4 * KC:NF * KC, :])

            def qk_proj(w_i, wt, outT, out_eng):
                oq = ob.tile([128, KC, R], F8, name=f"oq{w_i}")
                for f in range(NF):
                    pz = ps.tile([128, R], F32, name=f"pz{w_i}_{f}", tag="ps")
                    for kk in range(KC // 2):
                        nc.tensor.matmul(
                            pz[:],
                            wt[:, f * KC + 2 * kk: f * KC + 2 * kk + 2, :],
                            xt[:, 2 * kk:2 * kk + 2, :],
                            start=(kk == 0), stop=(kk == KC // 2 - 1),
                            perf_mode=DR,
                        )
                    nc.scalar.activation(oq[:, f, :], pz[:], Ident,
                                         bias=bqkt[:, w_i, f:f + 1])
                    if f % 2 == 1:
                        out_eng.dma_start(outT[:, f - 1:f + 1, :], oq[:, f - 1:f + 1, :])

            qk_proj(0, wqt, qT, nc.gpsimd)
            ov = ob.tile([128, R // 128, D], F8, name="ov")
            for i in range(R // 128):
                for fh in (1, 0):    # half 1 first: its weights land early
                    cols = slice(fh * 512, (fh + 1) * 512)
                    pz = ps.tile([128, R], F32, name=f"pv{i}_{fh}", tag="ps")
                    for kk in range(KC // 2):
                        nc.tensor.matmul(
                            pz[:],
                            xt[:, 2 * kk:2 * kk + 2, i * 128:(i + 1) * 128],
                            wvt[:, 2 * kk:2 * kk + 2, cols],
                            start=(kk == 0), stop=(kk == KC // 2 - 1),
                            perf_mode=DR,
                        )
                    nc.vector.tensor_tensor(ov[:, i, cols], pz[:], bvtt[:, cols],
                                            op=mybir.AluOpType.add)
                nc.gpsimd.dma_start(vO[i * 128:(i + 1) * 128, :], ov[:, i, :])
            qk_proj(1, wkt, kT, nc.scalar)
    nc.finalize()
    return nc


def _build_phase2():
    """Per core: anti-causal attention for its 512 query rows vs all 4096 keys,
    fp8 DoubleRow, 512-wide moving outputs.

    Stage 1 per key tile j: scores^T [128 keys, 512 q] in 4 DoubleRow matmuls,
    exp(scale*s) on the activation engine, masked into fp8 P^T pair tiles
    (iota generated on-device). Stage 2 per feature chunk: PV accumulates all
    16 j-pairs in PSUM; ones column of V_aug gives the denominator.

      qt [128, KC, R] fp8 (phase-1 qT verbatim)
      kt [128, J*KC, 128] fp8: [p, j*KC+k, c] = (k+bk)^T[128k+p, 128j+c]
      vi [128, J, VA] fp8: [p, j, c] = v_aug[128j+p, c]
      th [128, J] f32: [p, j] = 128j+p-512*core
    Outputs: rdT [128, NF, R] bf16 numerators, dn [2, R] bf16 (row 0 = denom).
    """
    nc = bacc.Bacc(None, target_bir_lowering=False)
    qt = nc.dram_tensor("qt", [128, KC, R], F8, kind="ExternalInput")
    kt = nc.dram_tensor("kt", [128, J * KC, 128], F8, kind="ExternalInput")
    vi = nc.dram_tensor("vi", [128, J, VA], F8, kind="ExternalInput")
    th = nc.dram_tensor("th", [128, J], F32, kind="ExternalInput")
    rdT = nc.dram_tensor("rdT", [128, NF, R], BF16, kind="ExternalOutput")
    dn = nc.dram_tensor("dn", [2, R], BF16, kind="ExternalOutput")
    with TileContext(nc) as tc:
        with tc.tile_pool(name="cst", bufs=1) as cst, \
             tc.tile_pool(name="kp", bufs=5) as kp, \
             tc.tile_pool(name="vp", bufs=4) as vp, \
             tc.tile_pool(name="sp", bufs=2, space="PSUM") as sp, \
             tc.tile_pool(name="ep", bufs=3) as ep, \
             tc.tile_pool(name="pp", bufs=J // 2) as pp, \
             tc.tile_pool(name="p2", bufs=3, space="PSUM") as p2, \
             tc.tile_pool(name="no", bufs=2) as no:
            tht = cst.tile([128, J], F32)
            qtt = cst.tile([128, KC, R], F8)
            # leading DMAs split across queues so the first matmul waits on
            # ~270KB total, in parallel streams
            nc.sync.dma_start(tht[:], th[:])
            nc.sync.dma_start(qtt[:, 0:2, :], qt[:, 0:2, :])
            nc.sync.dma_start(qtt[:, 2:4, :], qt[:, 2:4, :])
            nc.sync.dma_start(qtt[:, 4:8, :], qt[:, 4:8, :])
            # on-device iota: [p, i] = i  (int32 -> f32 for the mask compare)
            ioi = cst.tile([128, R], I32)
            iot = cst.tile([128, R], F32)
            nc.gpsimd.iota(ioi[:], pattern=[[1, R]], base=0, channel_multiplier=0)
            nc.gpsimd.tensor_copy(iot[:], ioi[:])
            vgs = [vp.tile([128, J // 4, VA], F8, name=f"vg{g}", tag="vg")
                   for g in range(4)]
            pts = [pp.tile([128, 2, R], F8, name=f"pt{t}", tag="pt")
                   for t in range(J // 2)]
            ktgs = []
            for g, tiles in enumerate(KT_GROUPS):
                j0, ng = tiles[0], len(tiles)
                ktg = kp.tile([128, 4 * KC, 128], F8, name=f"ktg{g}", tag="kt")
                eng = nc.scalar if g in KT_ON_SCALAR else nc.sync
                eng.dma_start(ktg[:, 0:ng * KC, :], kt[:, j0 * KC:(j0 + ng) * KC, :])
                ktgs.append(ktg)
            # V loads: g0-g2 on scalar (after its kt groups), g3 via gpsimd
            for gv in range(3):
                nc.scalar.dma_start(vgs[gv][:],
                                    vi[:, gv * (J // 4):(gv + 1) * (J // 4), :])
            nc.gpsimd.dma_start(vgs[3][:], vi[:, 3 * (J // 4):J, :])
            for g, tiles in enumerate(KT_GROUPS):
                j0, ng = tiles[0], len(tiles)
                ktg = ktgs[g]
                for jj in range(ng):
                    j = j0 + jj
                    s = sp.tile([128, R], F32, name=f"s{j}", tag="s")
                    for kk in range(KC // 2):
                        nc.tensor.matmul(
                            s[:],
                            ktg[:, jj * KC + 2 * kk: jj * KC + 2 * kk + 2, :],
                            qtt[:, 2 * kk:2 * kk + 2, :],
                            start=(kk == 0), stop=(kk == KC // 2 - 1),
                            perf_mode=DR,
                        )
                    ex = ep.tile([128, R], F32, name=f"e{j}", tag="e")
                    nc.scalar.activation(ex[:], s[:], Exp, scale=SC)
                    nc.vector.scalar_tensor_tensor(
                        pts[j // 2][:, j % 2, :], iot[:], tht[:, j:j + 1], ex[:],
                        op0=mybir.AluOpType.is_le, op1=mybir.AluOpType.mult,
                    )
            rd_all = cst.tile([128, NF, R], BF16)
            for n in range(NF + 1):
                c0, w = (n * 128, 128) if n < NF else (D, 2)
                pz = p2.tile([128, R], F32, name=f"pv{n}", tag="pv")
                for t in range(J // 2):
                    gv, tt = t // 4, t % 4
                    nc.tensor.matmul(
                        pz[:w, :],
                        vgs[gv][:, 2 * tt:2 * tt + 2, c0:c0 + w],
                        pts[t][:],
                        start=(t == 0), stop=(t == J // 2 - 1),
                        perf_mode=DR,
                    )
                if n < NF:
                    if n % 2 == 0:
                        nc.scalar.copy(rd_all[:, n, :], pz[:])
                    else:
                        nc.vector.tensor_copy(rd_all[:, n, :], pz[:])
                        eng = nc.gpsimd if n < NF - 1 else nc.sync
                        eng.dma_start(rdT[:, n - 1:n + 1, :], rd_all[:, n - 1:n + 1, :])
                else:
                    o = no.tile([128, R], BF16, name="odn")
                    nc.vector.tensor_copy(o[:2, :], pz[:2, :])
                    nc.sync.dma_start(dn[:], o[:2, :])
    nc.finalize()
    return nc


def _f8(a):
    return np.asarray(a, dtype=NP_F8)


def prep_phase1_inputs(x, Wq, Wk, Wv, bq, bk, bv):
    wq_in = _f8(Wq.reshape(KC, 128, NF, 128).transpose(1, 2, 0, 3)
                .reshape(128, NF * KC, 128))
    wk_in = _f8(Wk.reshape(KC, 128, NF, 128).transpose(1, 2, 0, 3)
                .reshape(128, NF * KC, 128))
    wv_in = _f8(Wv.reshape(KC, 128, D).transpose(1, 0, 2))
    bqk_in = np.ascontiguousarray(
        np.stack([bq.reshape(NF, 128).T, bk.reshape(NF, 128).T], axis=1)
    ).astype(np.float32)                               # [p, 2, f]
    bvt_in = np.ascontiguousarray(
        np.broadcast_to(bv, (128, D))).astype(ml_dtypes.bfloat16)
    in_maps = []
    for c in range(NCORES):
        xs = x[c * R:(c + 1) * R]                      # [R, D]
        xsT = _f8(xs.T.reshape(KC, 128, R).transpose(1, 0, 2))  # [p, k, i]
        in_maps.append({"xsT": xsT, "wq": wq_in, "wk": wk_in, "wv": wv_in,
                        "bqk": bqk_in, "bvt": bvt_in})
    return in_maps


def prep_phase2_inputs(res1):
    # kt/vi: pure byte permutations of the fp8 phase-1 outputs (bias included)
    kT_full = np.concatenate(
        [np.asarray(res1[c]["kT"]) for c in range(NCORES)], axis=2
    )                                                  # [128, KC, S] fp8
    kt_in = np.ascontiguousarray(
        kT_full.reshape(128, KC, J, 128).transpose(0, 2, 1, 3)
        .reshape(128, J * KC, 128))
    v_full = np.concatenate(
        [np.asarray(res1[c]["vO"]) for c in range(NCORES)], axis=0
    )                                                  # [S, D] fp8
    v_aug = np.concatenate(
        [v_full,
         np.ones((S, 1), NP_F8),
         np.zeros((S, VA - D - 1), NP_F8)], axis=1)
    vi_in = np.ascontiguousarray(v_aug.reshape(J, 128, VA).transpose(1, 0, 2))
    p_idx = np.arange(128, dtype=np.float32)[:, None]
    j_idx = np.arange(J, dtype=np.float32)[None, :]
    in_maps = []
    for c in range(NCORES):
        thr_c = np.ascontiguousarray(128.0 * j_idx + p_idx - 512.0 * c).astype(np.float32)
        in_maps.append({
            "qt": np.asarray(res1[c]["qT"]), "kt": kt_in, "vi": vi_in,
            "th": thr_c,
        })
    return in_maps


def finish_output(x, res2):
    read = np.concatenate(
        [
            (np.asarray(res2[c]["rdT"], dtype=np.float32).transpose(1, 0, 2)
             .reshape(D, R)
             / np.asarray(res2[c]["dn"], dtype=np.float32)[0:1, :]).T
            for c in range(NCORES)
        ],
        axis=0,
    )
    return np.concatenate([x, read], axis=1).astype(np.float32)


def kernel(x, Wk, bk, Wq, bq, Wv, bv):
    global LAST_NS, PHASE_NS
    x = np.asarray(x, dtype=np.float32)
    Wk = np.asarray(Wk, dtype=np.float32)
    Wq = np.asarray(Wq, dtype=np.float32)
    Wv = np.asarray(Wv, dtype=np.float32)
    bk = np.asarray(bk, dtype=np.float32)
    bq = np.asarray(bq, dtype=np.float32)
    bv = np.asarray(bv, dtype=np.float32)

    if "p1" not in _cache:
        _cache["p1"] = _build_phase1()
    if "p2" not in _cache:
        _cache["p2"] = _build_phase2()

    in_maps1 = prep_phase1_inputs(x, Wq, Wk, Wv, bq, bk, bv)
    r1 = run_bass_kernel_spmd(_cache["p1"], in_maps1, list(range(NCORES)), trace=TRACE)
    in_maps2 = prep_phase2_inputs(r1.results)
    r2 = run_bass_kernel_spmd(_cache["p2"], in_maps2, list(range(NCORES)), trace=TRACE)
    if TRACE and r1.exec_time_ns and r2.exec_time_ns:
        PHASE_NS = (int(r1.exec_time_ns), int(r2.exec_time_ns))
        LAST_NS = int(r1.exec_time_ns + r2.exec_time_ns)
    return finish_output(x, r2.results)


# revision 14
# speedup vs baseline: 1.0637x; 1.0637x over previous
import sys

sys.path.insert(0, "/opt/trn_rl_repo")

import math

import ml_dtypes
import numpy as np
import concourse.bass as bass  # noqa: F401  (registers types)
from concourse import bacc
import concourse.mybir as mybir
from concourse.tile import TileContext
from concourse.bass_utils import run_bass_kernel_spmd

S = 4096          # sequence length
D = 1024          # model/key/value dim
NCORES = 8
R = S // NCORES   # 512 query rows per core
KC = D // 128     # 8 contraction chunks
NF = D // 128     # 8 feature chunks
J = S // 128      # 32 key tiles
VA = D + 16       # V augmented with ones col (denominator); padded so the
                  # DoubleRow weight AP dim1 step stays %16==0
SC = 1.0 / math.sqrt(D)

F32 = mybir.dt.float32
I32 = mybir.dt.int32
BF16 = mybir.dt.bfloat16
F8 = mybir.dt.float8e4
DR = mybir.MatmulPerfMode.DoubleRow
Exp = mybir.ActivationFunctionType.Exp
Ident = mybir.ActivationFunctionType.Identity
NP_F8 = ml_dtypes.float8_e4m3

# phase-2 kt DMA groups: ramp up so the first matmul waits on 128KB only.
# The first two groups ride the scalar queue, in parallel with th/qt on sync.
KT_GROUPS = [[0], [1, 2, 3]] + [list(range(4 * g, 4 * g + 4)) for g in range(1, 8)]
KT_ON_SCALAR = {0, 1}

_cache = {}
# test.py can flip TRACE to get exec_time_ns of the two launches in LAST_NS
TRACE = False
LAST_NS = None
PHASE_NS = None


def _build_phase1():
    """Per core: q^T+bq, k^T+bk (transposed, contraction-chunked, fp8 out) and
    v+bv (natural, fp8) for its 512-row x slice. fp8 DoubleRow matmuls,
    512-wide moving output. Compute order q, v, k with outputs shipped as
    soon as produced so the post-compute DMA drain stays small.

      xsT [128, KC, R]: [p, k, i] = x[512c+i, 128k+p]
      wq/wk [128, NF*KC, 128]: [p, f*KC+k, c] = W[128k+p, 128f+c]
      wv [128, KC, D]: [p, k, d] = W[128k+p, d]
      bqk [128, 2, NF] f32: [p, 0/1, f] = bq/bk[128f+p]
      bvt [128, D] bf16: bv broadcast to all partitions
    Outputs (all biased on device, single fp8 quantization):
      qT/kT [128, KC, R] fp8: [p, f, i] = (x@W + b)^T[128f+p, i]
      vO [R, D] fp8 natural.
    """
    nc = bacc.Bacc(None, target_bir_lowering=False)
    xsT = nc.dram_tensor("xsT", [128, KC, R], F8, kind="ExternalInput")
    wq = nc.dram_tensor("wq", [128, NF * KC, 128], F8, kind="ExternalInput")
    wk = nc.dram_tensor("wk", [128, NF * KC, 128], F8, kind="ExternalInput")
    wv = nc.dram_tensor("wv", [128, KC, D], F8, kind="ExternalInput")
    bqk = nc.dram_tensor("bqk", [128, 2, NF], F32, kind="ExternalInput")
    bvt = nc.dram_tensor("bvt", [128, D], BF16, kind="ExternalInput")
    qT = nc.dram_tensor("qT", [128, KC, R], F8, kind="ExternalOutput")
    kT = nc.dram_tensor("kT", [128, KC, R], F8, kind="ExternalOutput")
    vO = nc.dram_tensor("vO", [R, D], F8, kind="ExternalOutput")
    with TileContext(nc) as tc:
        with tc.tile_pool(name="inp", bufs=1) as inp, \
             tc.tile_pool(name="ob", bufs=1) as ob, \
             tc.tile_pool(name="ps", bufs=4, space="PSUM") as ps:
            bqkt = inp.tile([128, 2, NF], F32)
            bvtt = inp.tile([128, D], BF16)
            xt = inp.tile([128, KC, R], F8)
            wqt = inp.tile([128, NF * KC, 128], F8)
            wkt = inp.tile([128, NF * KC, 128], F8)
            wvt = inp.tile([128, KC, D], F8)
            nc.scalar.dma_start(bqkt[:], bqk[:])
            nc.scalar.dma_start(bvtt[:], bvt[:])
            nc.sync.dma_start(xt[:, 0:2, :], xsT[:, 0:2, :])
            nc.sync.dma_start(wqt[:, 0:2 * KC, :], wq[:, 0:2 * KC, :])
            nc.sync.dma_start(xt[:, 2:4, :], xsT[:, 2:4, :])
            nc.sync.dma_start(wqt[:, 2 * KC:4 * KC, :], wq[:, 2 * KC:4 * KC, :])
            nc.sync.dma_start(xt[:, 4:8, :], xsT[:, 4:8, :])
            nc.sync.dma_start(wqt[:, 4 * KC:NF * KC, :], wq[:, 4 * KC:NF * KC, :])
            nc.sync.dma_start(wvt[:, 0:4, :], wv[:, 0:4, :])
            nc.sync.dma_start(wvt[:, 4:KC, :], wv[:, 4:KC, :])
            nc.sync.dma_start(wkt[:, 0:4 * KC, :], wk[:, 0:4 * KC, :])
            nc.sync.dma_start(wkt[:, 4 * KC:NF * KC, :], wk[:, 4 * KC:NF * KC, :])

            def qk_proj(w_i, wt, outT):
                oq = ob.tile([128, KC, R], F8, name=f"oq{w_i}")
                for f in range(NF):
                    pz = ps.tile([128, R], F32, name=f"pz{w_i}_{f}", tag="ps")
                    for kk in range(KC // 2):
                        nc.tensor.matmul(
                            pz[:],
                            wt[:, f * KC + 2 * kk: f * KC + 2 * kk + 2, :],
                            xt[:, 2 * kk:2 * kk + 2, :],
                            start=(kk == 0), stop=(kk == KC // 2 - 1),
                            perf_mode=DR,
                        )
                    nc.scalar.activation(oq[:, f, :], pz[:], Ident,
                                         bias=bqkt[:, w_i, f:f + 1])
                    if f == 3:
                        nc.scalar.dma_start(outT[:, 0:4, :], oq[:, 0:4, :])
                nc.scalar.dma_start(outT[:, 4:KC, :], oq[:, 4:KC, :])

            qk_proj(0, wqt, qT)
            ov = ob.tile([128, R // 128, D], F8, name="ov")
            for i in range(R // 128):
                for fh in range(2):
                    cols = slice(fh * 512, (fh + 1) * 512)
                    pz = ps.tile([128, R], F32, name=f"pv{i}_{fh}", tag="ps")
                    for kk in range(KC // 2):
                        nc.tensor.matmul(
                            pz[:],
                            xt[:, 2 * kk:2 * kk + 2, i * 128:(i + 1) * 128],
                            wvt[:, 2 * kk:2 * kk + 2, cols],
                            start=(kk == 0), stop=(kk == KC // 2 - 1),
                            perf_mode=DR,
                        )
                    nc.vector.tensor_tensor(ov[:, i, cols], pz[:], bvtt[:, cols],
                                            op=mybir.AluOpType.add)
                nc.scalar.dma_start(vO[i * 128:(i + 1) * 128, :], ov[:, i, :])
            qk_proj(1, wkt, kT)
    nc.finalize()
    return nc


def _build_phase2():
    """Per core: anti-causal attention for its 512 query rows vs all 4096 keys,
    fp8 DoubleRow, 512-wide moving outputs.

    Stage 1 per key tile j: scores^T [128 keys, 512 q] in 4 DoubleRow matmuls,
    exp(scale*s) on the activation engine, masked into fp8 P^T pair tiles
    (iota generated on-device). Stage 2 per feature chunk: PV accumulates all
    16 j-pairs in PSUM; ones column of V_aug gives the denominator.

      qt [128, KC, R] fp8 (phase-1 qT verbatim)
      kt [128, J*KC, 128] fp8: [p, j*KC+k, c] = (k+bk)^T[128k+p, 128j+c]
      vi [128, J, VA] fp8: [p, j, c] = v_aug[128j+p, c]
      th [128, J] f32: [p, j] = 128j+p-512*core
    Outputs: rdT [128, NF, R] bf16 numerators, dn [2, R] bf16 (row 0 = denom).
    """
    nc = bacc.Bacc(None, target_bir_lowering=False)
    qt = nc.dram_tensor("qt", [128, KC, R], F8, kind="ExternalInput")
    kt = nc.dram_tensor("kt", [128, J * KC, 128], F8, kind="ExternalInput")
    vi = nc.dram_tensor("vi", [128, J, VA], F8, kind="ExternalInput")
    th = nc.dram_tensor("th", [128, J], F32, kind="ExternalInput")
    rdT = nc.dram_tensor("rdT", [128, NF, R], BF16, kind="ExternalOutput")
    dn = nc.dram_tensor("dn", [2, R], BF16, kind="ExternalOutput")
    with TileContext(nc) as tc:
        with tc.tile_pool(name="cst", bufs=1) as cst, \
             tc.tile_pool(name="kp", bufs=5) as kp, \
             tc.tile_pool(name="vp", bufs=4) as vp, \
             tc.tile_pool(name="sp", bufs=2, space="PSUM") as sp, \
             tc.tile_pool(name="ep", bufs=3) as ep, \
             tc.tile_pool(name="pp", bufs=J // 2) as pp, \
             tc.tile_pool(name="p2", bufs=3, space="PSUM") as p2, \
             tc.tile_pool(name="no", bufs=2) as no:
            tht = cst.tile([128, J], F32)
            qtt = cst.tile([128, KC, R], F8)
            nc.sync.dma_start(tht[:], th[:])
            nc.sync.dma_start(qtt[:, 0:2, :], qt[:, 0:2, :])
            nc.sync.dma_start(qtt[:, 2:4, :], qt[:, 2:4, :])
            nc.sync.dma_start(qtt[:, 4:8, :], qt[:, 4:8, :])
            # on-device iota: [p, i] = i  (int32 -> f32 for the mask compare)
            ioi = cst.tile([128, R], I32)
            iot = cst.tile([128, R], F32)
            nc.gpsimd.iota(ioi[:], pattern=[[1, R]], base=0, channel_multiplier=0)
            nc.gpsimd.tensor_copy(iot[:], ioi[:])
            vgs = [vp.tile([128, J // 4, VA], F8, name=f"vg{g}", tag="vg")
                   for g in range(4)]
            pts = [pp.tile([128, 2, R], F8, name=f"pt{t}", tag="pt")
                   for t in range(J // 2)]
            for g, tiles in enumerate(KT_GROUPS):
                j0, ng = tiles[0], len(tiles)
                ktg = kp.tile([128, 4 * KC, 128], F8, name=f"ktg{g}", tag="kt")
                eng = nc.scalar if g in KT_ON_SCALAR else nc.sync
                eng.dma_start(ktg[:, 0:ng * KC, :], kt[:, j0 * KC:(j0 + ng) * KC, :])
                # V loads ride the scalar queue, interleaved mid-stage-1 so
                # they neither delay kt tiles nor arrive late for PV
                if 2 <= g <= 5:
                    gv = g - 2
                    nc.scalar.dma_start(vgs[gv][:],
                                        vi[:, gv * (J // 4):(gv + 1) * (J // 4), :])
                for jj in range(ng):
                    j = j0 + jj
                    s = sp.tile([128, R], F32, name=f"s{j}", tag="s")
                    for kk in range(KC // 2):
                        nc.tensor.matmul(
                            s[:],
                            ktg[:, jj * KC + 2 * kk: jj * KC + 2 * kk + 2, :],
                            qtt[:, 2 * kk:2 * kk + 2, :],
                            start=(kk == 0), stop=(kk == KC // 2 - 1),
                            perf_mode=DR,
                        )
                    ex = ep.tile([128, R], F32, name=f"e{j}", tag="e")
                    nc.scalar.activation(ex[:], s[:], Exp, scale=SC)
                    nc.vector.scalar_tensor_tensor(
                        pts[j // 2][:, j % 2, :], iot[:], tht[:, j:j + 1], ex[:],
                        op0=mybir.AluOpType.is_le, op1=mybir.AluOpType.mult,
                    )
            rd_all = cst.tile([128, NF, R], BF16)
            for n in range(NF + 1):
                c0, w = (n * 128, 128) if n < NF else (D, 2)
                pz = p2.tile([128, R], F32, name=f"pv{n}", tag="pv")
                for t in range(J // 2):
                    gv, tt = t // 4, t % 4
                    nc.tensor.matmul(
                        pz[:w, :],
                        vgs[gv][:, 2 * tt:2 * tt + 2, c0:c0 + w],
                        pts[t][:],
                        start=(t == 0), stop=(t == J // 2 - 1),
                        perf_mode=DR,
                    )
                if n < NF:
                    if n % 2 == 0:
                        nc.scalar.copy(rd_all[:, n, :], pz[:])
                    else:
                        nc.vector.tensor_copy(rd_all[:, n, :], pz[:])
                    if n == 3:
                        nc.sync.dma_start(rdT[:, 0:4, :], rd_all[:, 0:4, :])
                else:
                    o = no.tile([128, R], BF16, name="odn")
                    nc.vector.tensor_copy(o[:2, :], pz[:2, :])
                    nc.sync.dma_start(dn[:], o[:2, :])
            nc.sync.dma_start(rdT[:, 4:NF, :], rd_all[:, 4:NF, :])
    nc.finalize()
    return nc


def _f8(a):
    return np.asarray(a, dtype=NP_F8)


def prep_phase1_inputs(x, Wq, Wk, Wv, bq, bk, bv):
    wq_in = _f8(Wq.reshape(KC, 128, NF, 128).transpose(1, 2, 0, 3)
                .reshape(128, NF * KC, 128))
    wk_in = _f8(Wk.reshape(KC, 128, NF, 128).transpose(1, 2, 0, 3)
                .reshape(128, NF * KC, 128))
    wv_in = _f8(Wv.reshape(KC, 128, D).transpose(1, 0, 2))
    bqk_in = np.ascontiguousarray(
        np.stack([bq.reshape(NF, 128).T, bk.reshape(NF, 128).T], axis=1)
    ).astype(np.float32)                               # [p, 2, f]
    bvt_in = np.ascontiguousarray(
        np.broadcast_to(bv, (128, D))).astype(ml_dtypes.bfloat16)
    in_maps = []
    for c in range(NCORES):
        xs = x[c * R:(c + 1) * R]                      # [R, D]
        xsT = _f8(xs.T.reshape(KC, 128, R).transpose(1, 0, 2))  # [p, k, i]
        in_maps.append({"xsT": xsT, "wq": wq_in, "wk": wk_in, "wv": wv_in,
                        "bqk": bqk_in, "bvt": bvt_in})
    return in_maps


def prep_phase2_inputs(res1):
    # kt/vi: pure byte permutations of the fp8 phase-1 outputs (bias included)
    kT_full = np.concatenate(
        [np.asarray(res1[c]["kT"]) for c in range(NCORES)], axis=2
    )                                                  # [128, KC, S] fp8
    kt_in = np.ascontiguousarray(
        kT_full.reshape(128, KC, J, 128).transpose(0, 2, 1, 3)
        .reshape(128, J * KC, 128))
    v_full = np.concatenate(
        [np.asarray(res1[c]["vO"]) for c in range(NCORES)], axis=0
    )                                                  # [S, D] fp8
    v_aug = np.concatenate(
        [v_full,
         np.ones((S, 1), NP_F8),
         np.zeros((S, VA - D - 1), NP_F8)], axis=1)
    vi_in = np.ascontiguousarray(v_aug.reshape(J, 128, VA).transpose(1, 0, 2))
    p_idx = np.arange(128, dtype=np.float32)[:, None]
    j_idx = np.arange(J, dtype=np.float32)[None, :]
    in_maps = []
    for c in range(NCORES):
        thr_c = np.ascontiguousarray(128.0 * j_idx + p_idx - 512.0 * c).astype(np.float32)
        in_maps.append({
            "qt": np.asarray(res1[c]["qT"]), "kt": kt_in, "vi": vi_in,
            "th": thr_c,
        })
    return in_maps


def finish_output(x, res2):
    read = np.concatenate(
        [
            (np.asarray(res2[c]["rdT"], dtype=np.float32).transpose(1, 0, 2)
             .reshape(D, R)
             / np.asarray(res2[c]["dn"], dtype=np.float32)[0:1, :]).T
            for c in range(NCORES)
        ],
        axis=0,
    )
    return np.concatenate([x, read], axis=1).astype(np.float32)


def kernel(x, Wk, bk, Wq, bq, Wv, bv):
    global LAST_NS, PHASE_NS
    x = np.asarray(x, dtype=np.float32)
    Wk = np.asarray(Wk, dtype=np.float32)
    Wq = np.asarray(Wq, dtype=np.float32)
    Wv = np.asarray(Wv, dtype=np.float32)
    bk = np.asarray(bk, dtype=np.float32)
    bq = np.asarray(bq, dtype=np.float32)
    bv = np.asarray(bv, dtype=np.float32)

    if "p1" not in _cache:
        _cache["p1"] = _build_phase1()
    if "p2" not in _cache:
        _cache["p2"] = _build_phase2()

    in_maps1 = prep_phase1_inputs(x, Wq, Wk, Wv, bq, bk, bv)
    r1 = run_bass_kernel_spmd(_cache["p1"], in_maps1, list(range(NCORES)), trace=TRACE)
    in_maps2 = prep_phase2_inputs(r1.results)
    r2 = run_bass_kernel_spmd(_cache["p2"], in_maps2, list(range(NCORES)), trace=TRACE)
    if TRACE and r1.exec_time_ns and r2.exec_time_ns:
        PHASE_NS = (int(r1.exec_time_ns), int(r2.exec_time_ns))
        LAST_NS = int(r1.exec_time_ns + r2.exec_time_ns)
    return finish_output(x, r2.results)


# revision 15
# speedup vs baseline: 1.0645x; 1.0008x over previous
import sys

sys.path.insert(0, "/opt/trn_rl_repo")

import math

import ml_dtypes
import numpy as np
import concourse.bass as bass  # noqa: F401  (registers types)
from concourse import bacc
import concourse.mybir as mybir
from concourse.tile import TileContext
from concourse.bass_utils import run_bass_kernel_spmd

S = 4096          # sequence length
D = 1024          # model/key/value dim
NCORES = 8
R = S // NCORES   # 512 query rows per core
KC = D // 128     # 8 contraction chunks
NF = D // 128     # 8 feature chunks
J = S // 128      # 32 key tiles
VA = D + 16       # V augmented with ones col (denominator); padded so the
                  # DoubleRow weight AP dim1 step stays %16==0
SC = 1.0 / math.sqrt(D)

F32 = mybir.dt.float32
I32 = mybir.dt.int32
BF16 = mybir.dt.bfloat16
F8 = mybir.dt.float8e4
DR = mybir.MatmulPerfMode.DoubleRow
Exp = mybir.ActivationFunctionType.Exp
Ident = mybir.ActivationFunctionType.Identity
NP_F8 = ml_dtypes.float8_e4m3

# phase-2 kt DMA groups: ramp up so the first matmul waits on 128KB only.
KT_GROUPS = [[0], [1, 2, 3]] + [list(range(4 * g, 4 * g + 4)) for g in range(1, 8)]
KT_ON_SCALAR = set()

_cache = {}
# test.py can flip TRACE to get exec_time_ns of the two launches in LAST_NS
TRACE = False
LAST_NS = None
PHASE_NS = None


def _build_phase1():
    """Per core: q^T+bq, k^T+bk (transposed, contraction-chunked, fp8 out) and
    v+bv (natural, fp8) for its 512-row x slice. fp8 DoubleRow matmuls,
    512-wide moving output. Compute order q, v, k with outputs shipped as
    soon as produced so the post-compute DMA drain stays small.

      xsT [128, KC, R]: [p, k, i] = x[512c+i, 128k+p]
      wq/wk [128, NF*KC, 128]: [p, f*KC+k, c] = W[128k+p, 128f+c]
      wv [128, KC, D]: [p, k, d] = W[128k+p, d]
      bqk [128, 2, NF] f32: [p, 0/1, f] = bq/bk[128f+p]
      bvt [128, D] bf16: bv broadcast to all partitions
    Outputs (all biased on device, single fp8 quantization):
      qT/kT [128, KC, R] fp8: [p, f, i] = (x@W + b)^T[128f+p, i]
      vO [R, D] fp8 natural.
    """
    nc = bacc.Bacc(None, target_bir_lowering=False)
    xsT = nc.dram_tensor("xsT", [128, KC, R], F8, kind="ExternalInput")
    wq = nc.dram_tensor("wq", [128, NF * KC, 128], F8, kind="ExternalInput")
    wk = nc.dram_tensor("wk", [128, NF * KC, 128], F8, kind="ExternalInput")
    wv = nc.dram_tensor("wv", [128, KC, D], F8, kind="ExternalInput")
    bqk = nc.dram_tensor("bqk", [128, 2, NF], F32, kind="ExternalInput")
    bvt = nc.dram_tensor("bvt", [128, D], BF16, kind="ExternalInput")
    qT = nc.dram_tensor("qT", [128, KC, R], F8, kind="ExternalOutput")
    kT = nc.dram_tensor("kT", [128, KC, R], F8, kind="ExternalOutput")
    vO = nc.dram_tensor("vO", [R, D], F8, kind="ExternalOutput")
    with TileContext(nc) as tc:
        with tc.tile_pool(name="inp", bufs=1) as inp, \
             tc.tile_pool(name="ob", bufs=1) as ob, \
             tc.tile_pool(name="ps", bufs=4, space="PSUM") as ps:
            bqkt = inp.tile([128, 2, NF], F32)
            bvtt = inp.tile([128, D], BF16)
            xt = inp.tile([128, KC, R], F8)
            wqt = inp.tile([128, NF * KC, 128], F8)
            wkt = inp.tile([128, NF * KC, 128], F8)
            wvt = inp.tile([128, KC, D], F8)
            nc.scalar.dma_start(bqkt[:], bqk[:])
            nc.scalar.dma_start(bvtt[:], bvt[:])
            nc.sync.dma_start(xt[:, 0:2, :], xsT[:, 0:2, :])
            nc.sync.dma_start(wqt[:, 0:2 * KC, :], wq[:, 0:2 * KC, :])
            nc.sync.dma_start(xt[:, 2:4, :], xsT[:, 2:4, :])
            nc.sync.dma_start(wqt[:, 2 * KC:4 * KC, :], wq[:, 2 * KC:4 * KC, :])
            nc.sync.dma_start(xt[:, 4:8, :], xsT[:, 4:8, :])
            nc.sync.dma_start(wqt[:, 4 * KC:NF * KC, :], wq[:, 4 * KC:NF * KC, :])
            nc.sync.dma_start(wvt[:, 0:4, :], wv[:, 0:4, :])
            nc.sync.dma_start(wvt[:, 4:KC, :], wv[:, 4:KC, :])
            nc.sync.dma_start(wkt[:, 0:4 * KC, :], wk[:, 0:4 * KC, :])
            nc.sync.dma_start(wkt[:, 4 * KC:NF * KC, :], wk[:, 4 * KC:NF * KC, :])

            def qk_proj(w_i, wt, outT):
                oq = ob.tile([128, KC, R], F8, name=f"oq{w_i}")
                for f in range(NF):
                    pz = ps.tile([128, R], F32, name=f"pz{w_i}_{f}", tag="ps")
                    for kk in range(KC // 2):
                        nc.tensor.matmul(
                            pz[:],
                            wt[:, f * KC + 2 * kk: f * KC + 2 * kk + 2, :],
                            xt[:, 2 * kk:2 * kk + 2, :],
                            start=(kk == 0), stop=(kk == KC // 2 - 1),
                            perf_mode=DR,
                        )
                    nc.scalar.activation(oq[:, f, :], pz[:], Ident,
                                         bias=bqkt[:, w_i, f:f + 1])
                    if f == 3:
                        nc.scalar.dma_start(outT[:, 0:4, :], oq[:, 0:4, :])
                nc.scalar.dma_start(outT[:, 4:KC, :], oq[:, 4:KC, :])

            qk_proj(0, wqt, qT)
            ov = ob.tile([128, R // 128, D], F8, name="ov")
            for i in range(R // 128):
                for fh in range(2):
                    cols = slice(fh * 512, (fh + 1) * 512)
                    pz = ps.tile([128, R], F32, name=f"pv{i}_{fh}", tag="ps")
                    for kk in range(KC // 2):
                        nc.tensor.matmul(
                            pz[:],
                            xt[:, 2 * kk:2 * kk + 2, i * 128:(i + 1) * 128],
                            wvt[:, 2 * kk:2 * kk + 2, cols],
                            start=(kk == 0), stop=(kk == KC // 2 - 1),
                            perf_mode=DR,
                        )
                    nc.vector.tensor_tensor(ov[:, i, cols], pz[:], bvtt[:, cols],
                                            op=mybir.AluOpType.add)
                nc.scalar.dma_start(vO[i * 128:(i + 1) * 128, :], ov[:, i, :])
            qk_proj(1, wkt, kT)
    nc.finalize()
    return nc


def _build_phase2():
    """Per core: anti-causal attention for its 512 query rows vs all 4096 keys,
    fp8 DoubleRow, 512-wide moving outputs.

    Stage 1 per key tile j: scores^T [128 keys, 512 q] in 4 DoubleRow matmuls,
    exp(scale*s) on the activation engine, masked into fp8 P^T pair tiles
    (iota generated on-device). Stage 2 per feature chunk: PV accumulates all
    16 j-pairs in PSUM; ones column of V_aug gives the denominator.

      qt [128, KC, R] fp8 (phase-1 qT verbatim)
      kt [128, J*KC, 128] fp8: [p, j*KC+k, c] = (k+bk)^T[128k+p, 128j+c]
      vi [128, J, VA] fp8: [p, j, c] = v_aug[128j+p, c]
      th [128, J] f32: [p, j] = 128j+p-512*core
    Outputs: rdT [128, NF, R] bf16 numerators, dn [2, R] bf16 (row 0 = denom).
    """
    nc = bacc.Bacc(None, target_bir_lowering=False)
    qt = nc.dram_tensor("qt", [128, KC, R], F8, kind="ExternalInput")
    kt = nc.dram_tensor("kt", [128, J * KC, 128], F8, kind="ExternalInput")
    vi = nc.dram_tensor("vi", [128, J, VA], F8, kind="ExternalInput")
    th = nc.dram_tensor("th", [128, J], F32, kind="ExternalInput")
    rdT = nc.dram_tensor("rdT", [128, NF, R], BF16, kind="ExternalOutput")
    dn = nc.dram_tensor("dn", [2, R], BF16, kind="ExternalOutput")
    with TileContext(nc) as tc:
        with tc.tile_pool(name="cst", bufs=1) as cst, \
             tc.tile_pool(name="kp", bufs=5) as kp, \
             tc.tile_pool(name="vp", bufs=4) as vp, \
             tc.tile_pool(name="sp", bufs=2, space="PSUM") as sp, \
             tc.tile_pool(name="ep", bufs=3) as ep, \
             tc.tile_pool(name="pp", bufs=J // 2) as pp, \
             tc.tile_pool(name="p2", bufs=3, space="PSUM") as p2, \
             tc.tile_pool(name="no", bufs=2) as no:
            tht = cst.tile([128, J], F32)
            qtt = cst.tile([128, KC, R], F8)
            nc.sync.dma_start(tht[:], th[:])
            nc.sync.dma_start(qtt[:, 0:2, :], qt[:, 0:2, :])
            nc.sync.dma_start(qtt[:, 2:4, :], qt[:, 2:4, :])
            nc.sync.dma_start(qtt[:, 4:8, :], qt[:, 4:8, :])
            # on-device iota: [p, i] = i  (int32 -> f32 for the mask compare)
            ioi = cst.tile([128, R], I32)
            iot = cst.tile([128, R], F32)
            nc.gpsimd.iota(ioi[:], pattern=[[1, R]], base=0, channel_multiplier=0)
            nc.gpsimd.tensor_copy(iot[:], ioi[:])
            vgs = [vp.tile([128, J // 4, VA], F8, name=f"vg{g}", tag="vg")
                   for g in range(4)]
            pts = [pp.tile([128, 2, R], F8, name=f"pt{t}", tag="pt")
                   for t in range(J // 2)]
            for g, tiles in enumerate(KT_GROUPS):
                j0, ng = tiles[0], len(tiles)
                ktg = kp.tile([128, 4 * KC, 128], F8, name=f"ktg{g}", tag="kt")
                eng = nc.scalar if g in KT_ON_SCALAR else nc.sync
                eng.dma_start(ktg[:, 0:ng * KC, :], kt[:, j0 * KC:(j0 + ng) * KC, :])
                # V loads ride the scalar queue, interleaved mid-stage-1 so
                # they neither delay kt tiles nor arrive late for PV
                if 2 <= g <= 5:
                    gv = g - 2
                    nc.scalar.dma_start(vgs[gv][:],
                                        vi[:, gv * (J // 4):(gv + 1) * (J // 4), :])
                for jj in range(ng):
                    j = j0 + jj
                    s = sp.tile([128, R], F32, name=f"s{j}", tag="s")
                    for kk in range(KC // 2):
                        nc.tensor.matmul(
                            s[:],
                            ktg[:, jj * KC + 2 * kk: jj * KC + 2 * kk + 2, :],
                            qtt[:, 2 * kk:2 * kk + 2, :],
                            start=(kk == 0), stop=(kk == KC // 2 - 1),
                            perf_mode=DR,
                        )
                    ex = ep.tile([128, R], F32, name=f"e{j}", tag="e")
                    nc.scalar.activation(ex[:], s[:], Exp, scale=SC)
                    nc.vector.scalar_tensor_tensor(
                        pts[j // 2][:, j % 2, :], iot[:], tht[:, j:j + 1], ex[:],
                        op0=mybir.AluOpType.is_le, op1=mybir.AluOpType.mult,
                    )
            rd_all = cst.tile([128, NF, R], BF16)
            for n in range(NF + 1):
                c0, w = (n * 128, 128) if n < NF else (D, 2)
                pz = p2.tile([128, R], F32, name=f"pv{n}", tag="pv")
                for t in range(J // 2):
                    gv, tt = t // 4, t % 4
                    nc.tensor.matmul(
                        pz[:w, :],
                        vgs[gv][:, 2 * tt:2 * tt + 2, c0:c0 + w],
                        pts[t][:],
                        start=(t == 0), stop=(t == J // 2 - 1),
                        perf_mode=DR,
                    )
                if n < NF:
                    if n % 2 == 0:
                        nc.scalar.copy(rd_all[:, n, :], pz[:])
                    else:
                        nc.vector.tensor_copy(rd_all[:, n, :], pz[:])
                    if n == 3:
                        nc.sync.dma_start(rdT[:, 0:4, :], rd_all[:, 0:4, :])
                else:
                    o = no.tile([128, R], BF16, name="odn")
                    nc.vector.tensor_copy(o[:2, :], pz[:2, :])
                    nc.sync.dma_start(dn[:], o[:2, :])
            nc.sync.dma_start(rdT[:, 4:NF, :], rd_all[:, 4:NF, :])
    nc.finalize()
    return nc


def _f8(a):
    return np.asarray(a, dtype=NP_F8)


def prep_phase1_inputs(x, Wq, Wk, Wv, bq, bk, bv):
    wq_in = _f8(Wq.reshape(KC, 128, NF, 128).transpose(1, 2, 0, 3)
                .reshape(128, NF * KC, 128))
    wk_in = _f8(Wk.reshape(KC, 128, NF, 128).transpose(1, 2, 0, 3)
                .reshape(128, NF * KC, 128))
    wv_in = _f8(Wv.reshape(KC, 128, D).transpose(1, 0, 2))
    bqk_in = np.ascontiguousarray(
        np.stack([bq.reshape(NF, 128).T, bk.reshape(NF, 128).T], axis=1)
    ).astype(np.float32)                               # [p, 2, f]
    bvt_in = np.ascontiguousarray(
        np.broadcast_to(bv, (128, D))).astype(ml_dtypes.bfloat16)
    in_maps = []
    for c in range(NCORES):
        xs = x[c * R:(c + 1) * R]                      # [R, D]
        xsT = _f8(xs.T.reshape(KC, 128, R).transpose(1, 0, 2))  # [p, k, i]
        in_maps.append({"xsT": xsT, "wq": wq_in, "wk": wk_in, "wv": wv_in,
                        "bqk": bqk_in, "bvt": bvt_in})
    return in_maps


def prep_phase2_inputs(res1):
    # kt/vi: pure byte permutations of the fp8 phase-1 outputs (bias included)
    kT_full = np.concatenate(
        [np.asarray(res1[c]["kT"]) for c in range(NCORES)], axis=2
    )                                                  # [128, KC, S] fp8
    kt_in = np.ascontiguousarray(
        kT_full.reshape(128, KC, J, 128).transpose(0, 2, 1, 3)
        .reshape(128, J * KC, 128))
    v_full = np.concatenate(
        [np.asarray(res1[c]["vO"]) for c in range(NCORES)], axis=0
    )                                                  # [S, D] fp8
    v_aug = np.concatenate(
        [v_full,
         np.ones((S, 1), NP_F8),
         np.zeros((S, VA - D - 1), NP_F8)], axis=1)
    vi_in = np.ascontiguousarray(v_aug.reshape(J, 128, VA).transpose(1, 0, 2))
    p_idx = np.arange(128, dtype=np.float32)[:, None]
    j_idx = np.arange(J, dtype=np.float32)[None, :]
    in_maps = []
    for c in range(NCORES):
        thr_c = np.ascontiguousarray(128.0 * j_idx + p_idx - 512.0 * c).astype(np.float32)
        in_maps.append({
            "qt": np.asarray(res1[c]["qT"]), "kt": kt_in, "vi": vi_in,
            "th": thr_c,
        })
    return in_maps


def finish_output(x, res2):
    read = np.concatenate(
        [
            (np.asarray(res2[c]["rdT"], dtype=np.float32).transpose(1, 0, 2)
             .reshape(D, R)
             / np.asarray(res2[c]["dn"], dtype=np.float32)[0:1, :]).T
            for c in range(NCORES)
        ],
        axis=0,
    )
    return np.concatenate([x, read], axis=1).astype(np.float32)


def kernel(x, Wk, bk, Wq, bq, Wv, bv):
    global LAST_NS, PHASE_NS
    x = np.asarray(x, dtype=np.float32)
    Wk = np.asarray(Wk, dtype=np.float32)
    Wq = np.asarray(Wq, dtype=np.float32)
    Wv = np.asarray(Wv, dtype=np.float32)
    bk = np.asarray(bk, dtype=np.float32)
    bq = np.asarray(bq, dtype=np.float32)
    bv = np.asarray(bv, dtype=np.float32)

    if "p1" not in _cache:
        _cache["p1"] = _build_phase1()
    if "p2" not in _cache:
        _cache["p2"] = _build_phase2()

    in_maps1 = prep_phase1_inputs(x, Wq, Wk, Wv, bq, bk, bv)
    r1 = run_bass_kernel_spmd(_cache["p1"], in_maps1, list(range(NCORES)), trace=TRACE)
    in_maps2 = prep_phase2_inputs(r1.results)
    r2 = run_bass_kernel_spmd(_cache["p2"], in_maps2, list(range(NCORES)), trace=TRACE)
    if TRACE and r1.exec_time_ns and r2.exec_time_ns:
        PHASE_NS = (int(r1.exec_time_ns), int(r2.exec_time_ns))
        LAST_NS = int(r1.exec_time_ns + r2.exec_time_ns)
    return finish_output(x, r2.results)


# revision 16
# speedup vs baseline: 1.0721x; 1.0071x over previous
import sys

sys.path.insert(0, "/opt/trn_rl_repo")

import math

import ml_dtypes
import numpy as np
import concourse.bass as bass  # noqa: F401  (registers types)
from concourse import bacc
import concourse.mybir as mybir
from concourse.tile import TileContext
from concourse.bass_utils import run_bass_kernel_spmd

S = 4096          # sequence length
D = 1024          # model/key/value dim
NCORES = 8
R = S // NCORES   # 512 query rows per core
KC = D // 128     # 8 contraction chunks
NF = D // 128     # 8 feature chunks
J = S // 128      # 32 key tiles
VA = D + 16       # V augmented with ones col (denominator); padded so the
                  # DoubleRow weight AP dim1 step stays %16==0
SC = 1.0 / math.sqrt(D)

F32 = mybir.dt.float32
I32 = mybir.dt.int32
BF16 = mybir.dt.bfloat16
F8 = mybir.dt.float8e4
DR = mybir.MatmulPerfMode.DoubleRow
Exp = mybir.ActivationFunctionType.Exp
Ident = mybir.ActivationFunctionType.Identity
NP_F8 = ml_dtypes.float8_e4m3

# phase-2 kt DMA groups: ramp up so the first matmul waits on 128KB only.
KT_GROUPS = [[0], [1, 2, 3]] + [list(range(4 * g, 4 * g + 4)) for g in range(1, 8)]
KT_ON_SCALAR = set()

_cache = {}
# test.py can flip TRACE to get exec_time_ns of the two launches in LAST_NS
TRACE = False
LAST_NS = None
PHASE_NS = None


def _build_phase1():
    """Per core: q^T+bq, k^T+bk (transposed, contraction-chunked, fp8 out) and
    v+bv (natural, fp8) for its 512-row x slice. fp8 DoubleRow matmuls,
    512-wide moving output. Compute order q, v, k with outputs shipped as
    soon as produced so the post-compute DMA drain stays small.

      xsT [128, KC, R]: [p, k, i] = x[512c+i, 128k+p]
      wq/wk [128, NF*KC, 128]: [p, f*KC+k, c] = W[128k+p, 128f+c]
      wv [128, KC, D]: [p, k, d] = W[128k+p, d]
      bqk [128, 2, NF] f32: [p, 0/1, f] = bq/bk[128f+p]
      bvt [128, D] bf16: bv broadcast to all partitions
    Outputs (all biased on device, single fp8 quantization):
      qT/kT [128, KC, R] fp8: [p, f, i] = (x@W + b)^T[128f+p, i]
      vO [R, D] fp8 natural.
    """
    nc = bacc.Bacc(None, target_bir_lowering=False)
    xsT = nc.dram_tensor("xsT", [128, KC, R], F8, kind="ExternalInput")
    wq = nc.dram_tensor("wq", [128, NF * KC, 128], F8, kind="ExternalInput")
    wk = nc.dram_tensor("wk", [128, NF * KC, 128], F8, kind="ExternalInput")
    wv = nc.dram_tensor("wv", [128, KC, D], F8, kind="ExternalInput")
    bqk = nc.dram_tensor("bqk", [128, 2, NF], F32, kind="ExternalInput")
    bvt = nc.dram_tensor("bvt", [128, D], BF16, kind="ExternalInput")
    qT = nc.dram_tensor("qT", [128, KC, R], F8, kind="ExternalOutput")
    kT = nc.dram_tensor("kT", [128, KC, R], F8, kind="ExternalOutput")
    vO = nc.dram_tensor("vO", [R, D], F8, kind="ExternalOutput")
    with TileContext(nc) as tc:
        with tc.tile_pool(name="inp", bufs=1) as inp, \
             tc.tile_pool(name="ob", bufs=1) as ob, \
             tc.tile_pool(name="wp", bufs=1, space="PSUM") as wp, \
             tc.tile_pool(name="ps", bufs=4, space="PSUM") as ps:
            bqkt = inp.tile([128, 2, NF], F32)
            bvtt = inp.tile([128, D], BF16)
            xt = inp.tile([128, KC, R], F8)
            wqt = inp.tile([128, NF * KC, 128], F8)
            wkt = inp.tile([128, NF * KC, 128], F8)
            wvt = inp.tile([128, KC, D], F8)
            # PE warmup during the DMA head: ramps the tensor-engine p-state
            # so the first real matmuls run at full clock
            wup = inp.tile([128, 2, R], F8)
            nc.vector.memset(wup[:], 0)
            wps = wp.tile([128, R], F32, name="wps")
            for _ in range(6):
                nc.tensor.matmul(wps[:], wup[:, :, 0:128], wup[:],
                                 start=True, stop=True, perf_mode=DR)
            nc.scalar.dma_start(bqkt[:], bqk[:])
            nc.scalar.dma_start(xt[:, 0:2, :], xsT[:, 0:2, :])
            nc.sync.dma_start(wqt[:, 0:2 * KC, :], wq[:, 0:2 * KC, :])
            nc.scalar.dma_start(xt[:, 2:4, :], xsT[:, 2:4, :])
            nc.sync.dma_start(wqt[:, 2 * KC:4 * KC, :], wq[:, 2 * KC:4 * KC, :])
            nc.scalar.dma_start(xt[:, 4:8, :], xsT[:, 4:8, :])
            nc.scalar.dma_start(bvtt[:], bvt[:])
            nc.sync.dma_start(wqt[:, 4 * KC:NF * KC, :], wq[:, 4 * KC:NF * KC, :])
            nc.sync.dma_start(wvt[:, 0:4, :], wv[:, 0:4, :])
            nc.sync.dma_start(wvt[:, 4:KC, :], wv[:, 4:KC, :])
            nc.sync.dma_start(wkt[:, 0:4 * KC, :], wk[:, 0:4 * KC, :])
            nc.sync.dma_start(wkt[:, 4 * KC:NF * KC, :], wk[:, 4 * KC:NF * KC, :])

            def qk_proj(w_i, wt, outT):
                oq = ob.tile([128, KC, R], F8, name=f"oq{w_i}")
                for f in range(NF):
                    pz = ps.tile([128, R], F32, name=f"pz{w_i}_{f}", tag="ps")
                    for kk in range(KC // 2):
                        nc.tensor.matmul(
                            pz[:],
                            wt[:, f * KC + 2 * kk: f * KC + 2 * kk + 2, :],
                            xt[:, 2 * kk:2 * kk + 2, :],
                            start=(kk == 0), stop=(kk == KC // 2 - 1),
                            perf_mode=DR,
                        )
                    nc.scalar.activation(oq[:, f, :], pz[:], Ident,
                                         bias=bqkt[:, w_i, f:f + 1])
                    if f == 3:
                        nc.scalar.dma_start(outT[:, 0:4, :], oq[:, 0:4, :])
                nc.scalar.dma_start(outT[:, 4:KC, :], oq[:, 4:KC, :])

            qk_proj(0, wqt, qT)
            ov = ob.tile([128, R // 128, D], F8, name="ov")
            for i in range(R // 128):
                for fh in range(2):
                    cols = slice(fh * 512, (fh + 1) * 512)
                    pz = ps.tile([128, R], F32, name=f"pv{i}_{fh}", tag="ps")
                    for kk in range(KC // 2):
                        nc.tensor.matmul(
                            pz[:],
                            xt[:, 2 * kk:2 * kk + 2, i * 128:(i + 1) * 128],
                            wvt[:, 2 * kk:2 * kk + 2, cols],
                            start=(kk == 0), stop=(kk == KC // 2 - 1),
                            perf_mode=DR,
                        )
                    nc.vector.tensor_tensor(ov[:, i, cols], pz[:], bvtt[:, cols],
                                            op=mybir.AluOpType.add)
                nc.scalar.dma_start(vO[i * 128:(i + 1) * 128, :], ov[:, i, :])
            qk_proj(1, wkt, kT)
    nc.finalize()
    return nc


def _build_phase2():
    """Per core: anti-causal attention for its 512 query rows vs all 4096 keys,
    fp8 DoubleRow, 512-wide moving outputs.

    Stage 1 per key tile j: scores^T [128 keys, 512 q] in 4 DoubleRow matmuls,
    exp(scale*s) on the activation engine, masked into fp8 P^T pair tiles
    (iota generated on-device). Stage 2 per feature chunk: PV accumulates all
    16 j-pairs in PSUM; ones column of V_aug gives the denominator.

      qt [128, KC, R] fp8 (phase-1 qT verbatim)
      kt [128, J*KC, 128] fp8: [p, j*KC+k, c] = (k+bk)^T[128k+p, 128j+c]
      vi [128, J, VA] fp8: [p, j, c] = v_aug[128j+p, c]
      th [128, J] f32: [p, j] = 128j+p-512*core
    Outputs: rdT [128, NF, R] bf16 numerators, dn [2, R] bf16 (row 0 = denom).
    """
    nc = bacc.Bacc(None, target_bir_lowering=False)
    qt = nc.dram_tensor("qt", [128, KC, R], F8, kind="ExternalInput")
    kt = nc.dram_tensor("kt", [128, J * KC, 128], F8, kind="ExternalInput")
    vi = nc.dram_tensor("vi", [128, J, VA], F8, kind="ExternalInput")
    th = nc.dram_tensor("th", [128, J], F32, kind="ExternalInput")
    rdT = nc.dram_tensor("rdT", [128, NF, R], BF16, kind="ExternalOutput")
    dn = nc.dram_tensor("dn", [2, R], BF16, kind="ExternalOutput")
    with TileContext(nc) as tc:
        with tc.tile_pool(name="cst", bufs=1) as cst, \
             tc.tile_pool(name="kp", bufs=5) as kp, \
             tc.tile_pool(name="vp", bufs=4) as vp, \
             tc.tile_pool(name="wp", bufs=1, space="PSUM") as wp, \
             tc.tile_pool(name="sp", bufs=3, space="PSUM") as sp, \
             tc.tile_pool(name="ep", bufs=3) as ep, \
             tc.tile_pool(name="pp", bufs=J // 2) as pp, \
             tc.tile_pool(name="p2", bufs=3, space="PSUM") as p2, \
             tc.tile_pool(name="no", bufs=2) as no:
            tht = cst.tile([128, J], F32)
            qtt = cst.tile([128, KC, R], F8)
            wup = cst.tile([128, 2, R], F8)
            nc.vector.memset(wup[:], 0)
            wps = wp.tile([128, R], F32, name="wps2")
            for _ in range(6):
                nc.tensor.matmul(wps[:], wup[:, :, 0:128], wup[:],
                                 start=True, stop=True, perf_mode=DR)
            nc.sync.dma_start(tht[:], th[:])
            nc.sync.dma_start(qtt[:, 0:2, :], qt[:, 0:2, :])
            nc.sync.dma_start(qtt[:, 2:4, :], qt[:, 2:4, :])
            nc.sync.dma_start(qtt[:, 4:8, :], qt[:, 4:8, :])
            # on-device iota: [p, i] = i  (int32 -> f32 for the mask compare)
            ioi = cst.tile([128, R], I32)
            iot = cst.tile([128, R], F32)
            nc.gpsimd.iota(ioi[:], pattern=[[1, R]], base=0, channel_multiplier=0)
            nc.gpsimd.tensor_copy(iot[:], ioi[:])
            vgs = [vp.tile([128, J // 4, VA], F8, name=f"vg{g}", tag="vg")
                   for g in range(4)]
            pts = [pp.tile([128, 2, R], F8, name=f"pt{t}", tag="pt")
                   for t in range(J // 2)]
            for g, tiles in enumerate(KT_GROUPS):
                j0, ng = tiles[0], len(tiles)
                ktg = kp.tile([128, 4 * KC, 128], F8, name=f"ktg{g}", tag="kt")
                eng = nc.scalar if g in KT_ON_SCALAR else nc.sync
                eng.dma_start(ktg[:, 0:ng * KC, :], kt[:, j0 * KC:(j0 + ng) * KC, :])
                # V loads ride the scalar queue, interleaved mid-stage-1 so
                # they neither delay kt tiles nor arrive late for PV
                if 2 <= g <= 5:
                    gv = g - 2
                    nc.scalar.dma_start(vgs[gv][:],
                                        vi[:, gv * (J // 4):(gv + 1) * (J // 4), :])
                for jj in range(ng):
                    j = j0 + jj
                    s = sp.tile([128, R], F32, name=f"s{j}", tag="s")
                    for kk in range(KC // 2):
                        nc.tensor.matmul(
                            s[:],
                            ktg[:, jj * KC + 2 * kk: jj * KC + 2 * kk + 2, :],
                            qtt[:, 2 * kk:2 * kk + 2, :],
                            start=(kk == 0), stop=(kk == KC // 2 - 1),
                            perf_mode=DR,
                        )
                    ex = ep.tile([128, R], F32, name=f"e{j}", tag="e")
                    nc.scalar.activation(ex[:], s[:], Exp, scale=SC)
                    nc.vector.scalar_tensor_tensor(
                        pts[j // 2][:, j % 2, :], iot[:], tht[:, j:j + 1], ex[:],
                        op0=mybir.AluOpType.is_le, op1=mybir.AluOpType.mult,
                    )
            rd_all = cst.tile([128, NF, R], BF16)
            for n in range(NF + 1):
                c0, w = (n * 128, 128) if n < NF else (D, 2)
                pz = p2.tile([128, R], F32, name=f"pv{n}", tag="pv")
                for t in range(J // 2):
                    gv, tt = t // 4, t % 4
                    nc.tensor.matmul(
                        pz[:w, :],
                        vgs[gv][:, 2 * tt:2 * tt + 2, c0:c0 + w],
                        pts[t][:],
                        start=(t == 0), stop=(t == J // 2 - 1),
                        perf_mode=DR,
                    )
                if n < NF:
                    if n % 2 == 0:
                        nc.scalar.copy(rd_all[:, n, :], pz[:])
                    else:
                        nc.vector.tensor_copy(rd_all[:, n, :], pz[:])
                    if n == 3:
                        nc.sync.dma_start(rdT[:, 0:4, :], rd_all[:, 0:4, :])
                    if n == 5:
                        nc.sync.dma_start(rdT[:, 4:6, :], rd_all[:, 4:6, :])
                else:
                    o = no.tile([128, R], BF16, name="odn")
                    nc.vector.tensor_copy(o[:2, :], pz[:2, :])
                    nc.sync.dma_start(dn[:], o[:2, :])
            nc.sync.dma_start(rdT[:, 6:NF, :], rd_all[:, 6:NF, :])
    nc.finalize()
    return nc


def _f8(a):
    return np.asarray(a, dtype=NP_F8)


def prep_phase1_inputs(x, Wq, Wk, Wv, bq, bk, bv):
    wq_in = _f8(Wq.reshape(KC, 128, NF, 128).transpose(1, 2, 0, 3)
                .reshape(128, NF * KC, 128))
    wk_in = _f8(Wk.reshape(KC, 128, NF, 128).transpose(1, 2, 0, 3)
                .reshape(128, NF * KC, 128))
    wv_in = _f8(Wv.reshape(KC, 128, D).transpose(1, 0, 2))
    bqk_in = np.ascontiguousarray(
        np.stack([bq.reshape(NF, 128).T, bk.reshape(NF, 128).T], axis=1)
    ).astype(np.float32)                               # [p, 2, f]
    bvt_in = np.ascontiguousarray(
        np.broadcast_to(bv, (128, D))).astype(ml_dtypes.bfloat16)
    in_maps = []
    for c in range(NCORES):
        xs = x[c * R:(c + 1) * R]                      # [R, D]
        xsT = _f8(xs.T.reshape(KC, 128, R).transpose(1, 0, 2))  # [p, k, i]
        in_maps.append({"xsT": xsT, "wq": wq_in, "wk": wk_in, "wv": wv_in,
                        "bqk": bqk_in, "bvt": bvt_in})
    return in_maps


def prep_phase2_inputs(res1):
    # kt/vi: pure byte permutations of the fp8 phase-1 outputs (bias included)
    kT_full = np.concatenate(
        [np.asarray(res1[c]["kT"]) for c in range(NCORES)], axis=2
    )                                                  # [128, KC, S] fp8
    kt_in = np.ascontiguousarray(
        kT_full.reshape(128, KC, J, 128).transpose(0, 2, 1, 3)
        .reshape(128, J * KC, 128))
    v_full = np.concatenate(
        [np.asarray(res1[c]["vO"]) for c in range(NCORES)], axis=0
    )                                                  # [S, D] fp8
    v_aug = np.concatenate(
        [v_full,
         np.ones((S, 1), NP_F8),
         np.zeros((S, VA - D - 1), NP_F8)], axis=1)
    vi_in = np.ascontiguousarray(v_aug.reshape(J, 128, VA).transpose(1, 0, 2))
    p_idx = np.arange(128, dtype=np.float32)[:, None]
    j_idx = np.arange(J, dtype=np.float32)[None, :]
    in_maps = []
    for c in range(NCORES):
        thr_c = np.ascontiguousarray(128.0 * j_idx + p_idx - 512.0 * c).astype(np.float32)
        in_maps.append({
            "qt": np.asarray(res1[c]["qT"]), "kt": kt_in, "vi": vi_in,
            "th": thr_c,
        })
    return in_maps


def finish_output(x, res2):
    read = np.concatenate(
        [
            (np.asarray(res2[c]["rdT"], dtype=np.float32).transpose(1, 0, 2)
             .reshape(D, R)
             / np.asarray(res2[c]["dn"], dtype=np.float32)[0:1, :]).T
            for c in range(NCORES)
        ],
        axis=0,
    )
    return np.concatenate([x, read], axis=1).astype(np.float32)


def kernel(x, Wk, bk, Wq, bq, Wv, bv):
    global LAST_NS, PHASE_NS
    x = np.asarray(x, dtype=np.float32)
    Wk = np.asarray(Wk, dtype=np.float32)
    Wq = np.asarray(Wq, dtype=np.float32)
    Wv = np.asarray(Wv, dtype=np.float32)
    bk = np.asarray(bk, dtype=np.float32)
    bq = np.asarray(bq, dtype=np.float32)
    bv = np.asarray(bv, dtype=np.float32)

    if "p1" not in _cache:
        _cache["p1"] = _build_phase1()
    if "p2" not in _cache:
        _cache["p2"] = _build_phase2()

    in_maps1 = prep_phase1_inputs(x, Wq, Wk, Wv, bq, bk, bv)
    r1 = run_bass_kernel_spmd(_cache["p1"], in_maps1, list(range(NCORES)), trace=TRACE)
    in_maps2 = prep_phase2_inputs(r1.results)
    r2 = run_bass_kernel_spmd(_cache["p2"], in_maps2, list(range(NCORES)), trace=TRACE)
    if TRACE and r1.exec_time_ns and r2.exec_time_ns:
        PHASE_NS = (int(r1.exec_time_ns), int(r2.exec_time_ns))
        LAST_NS = int(r1.exec_time_ns + r2.exec_time_ns)
    return finish_output(x, r2.results)
